# revision 79
# baseline (speedup 1.0000x reference)
"""Additive attention (Bahdanau) Trainium2 kernel, data-parallel over 8 NeuronCores.

ref:
    enc_proj = einsum('bte,ie->bti', encoder_states, w1)
    dec_proj = einsum('bd,id->bi', decoder_state, w2)
    scores   = einsum('bti,oi->bt', tanh(enc_proj + dec_proj[:,None,:]), v)
    probs    = softmax(scores, axis=1)[:, :, None]
    ctx      = sum(probs * encoder_states, axis=1)
    returns (ctx, probs)

Sharding: batch (64) split 8-way across cores; the small weights are
replicated, host-packed into one constants blob in the transposed layouts the
TensorEngine wants (w1T, w2T, vT, identities).

Per-core dataflow (software-pipelined over the 8 local batches; all stages
emitted with explicit skews so each engine's in-order stream never stalls):
  - all HBM loads upfront: gpsimd SWDGE cast-DMA f32->bf16 in flight, with
    partition p holding 16 consecutive t rows (t = p*16 + j) so each
    partition's DRAM run is 16 KB contiguous (64 descriptors per half-batch)
  - transposes (2 batches ahead): PE is_transpose matmuls (bf16 -> PSUM-bf16
    blocks) + DVE copyback -> encT [e, t]. The DMA xbar is deliberately NOT
    used: Tile serializes every PASSTHROUGH<->TRANSPOSE xbar-mode switch
    across all queues (HW deadlock workaround) which throttles the loads.
  - phase1: PE bf16 matmuls (w1T stationary, encT moving) -> PSUM [i, t]
  - ScalarE tanh(psum + dec_proj[:, b] bias) -> SBUF bf16 [i, t]
  - scores (1 batch behind): PE matmul vT x tanh -> PSUM [1, 512] chunks;
    ScalarE exp writes the u row bf16 in t-order via a strided AP
    (+ accum_out chunk sums). |scores| < 2 so no max subtraction needed.
    Each chunk is immediately scattered to uT columns [t128, j] by a plain
    reshape DMA on the otherwise-idle sync HWDGE queue (contiguous runs).
  - phase3 (2 batches behind): PE matmul uT-column x enc-bf16 block, PSUM
    accumulated over the 16 t-blocks; DVE scales by 1/Z (normalize late)
    into ctx and probs staging; two output DMAs at the end.

Queue discipline (hard-won): gpsimd SWDGE = enc loads only (its FIFO must
never contain compute-gated DMAs), sync HWDGE = uT scatters, scalar = consts
+ outputs. A DMA must never be issued from the engine that computes its
source (same-engine issue emits no semaphore and races the ACT pipeline on
silicon).
"""

import numpy as np
import ml_dtypes

_B, _T, _E, _I, _D = 64, 2048, 256, 256, 256
_CBLOB_W = 4868
_NC = 8
_BL = _B // _NC  # batches per core

_cached = None


def _build():
    from contextlib import ExitStack

    import concourse.bass as bass
    import concourse.tile as tile
    from concourse import bacc, mybir

    f32 = mybir.dt.float32
    bf16 = mybir.dt.bfloat16
    AF = mybir.ActivationFunctionType

    nc = bacc.Bacc(
        "TRN2",
        target_bir_lowering=False,
        debug=False,
        num_devices=_NC,
        dynamic_dma_scratch_size=65536,
        num_swdge_queues=4,
    )

    u8 = mybir.dt.uint8
    enc = nc.declare_dram_parameter("enc", [_BL, _T, _E], f32, isOutput=False)
    cblob = nc.declare_dram_parameter("cblob", [128, _CBLOB_W], u8, isOutput=False)
    ctx_out = nc.declare_dram_parameter("ctx", [_BL, _E], f32, isOutput=True)
    probs_out = nc.declare_dram_parameter("probs", [_BL, _T], f32, isOutput=True)

    NTB = _T // 128  # 16 t-blocks per batch
    HALF_TB = NTB // 2  # load/cast/transpose granularity

    with tile.TileContext(nc) as tc, ExitStack() as ex:
        pool = lambda name, bufs, **kw: ex.enter_context(
            tc.tile_pool(name=name, bufs=bufs, **kw)
        )
        consts = pool("consts", 1)
        encbf_pool = pool("encbf", _BL)  # batch b's tile lives until phase3(b)
        encT_pool = pool("encT", 3)
        tanh_pool = pool("tanh", 2)
        small_pool = pool("small", 4)
        tail_pool = pool("tail", 1)
        p1_ps = pool("p1ps", 2, space="PSUM")  # [128, 1024] f32 -> 2 banks each
        tr_ps = pool("trps", 1, space="PSUM")  # [128, 8, 128] bf16 -> 1 bank each
        cx_ps = pool("cxps", 3, space="PSUM")  # scores/ctx/misc -> 1 bank each

        # ---------- constants: one host-packed blob, two DMAs (the first
        # carries just eye+dec so the dec-transpose preamble starts early) ----
        blob = consts.tile([128, _CBLOB_W], u8)
        nc.scalar.dma_start(blob[:, 0:1536], cblob.ap()[:, 0:1536])
        nc.scalar.dma_start(blob[:, 1536:], cblob.ap()[:, 1536:])
        eye_sb = blob[:, 0:512].bitcast(f32)  # [128, 128]
        dec_sb = blob[0:_BL, 512:1536].bitcast(f32)  # [8, 256]
        w2t_sb = blob[:, 1536:3584].bitcast(f32).rearrange(
            "p (dh i) -> p dh i", dh=2
        )  # (dp, dh, i)
        w1t_sb = blob[:, 3584:4608].bitcast(bf16).rearrange(
            "p (eh i) -> p eh i", eh=2
        )  # (ep, eh, i)
        eyebf_sb = blob[:, 4608:4864].bitcast(bf16)  # [128, 128]
        vt_sb = blob[:, 4864:4868].bitcast(bf16).rearrange(
            "p (ih o) -> p ih o", ih=2
        )  # (ip, ih, 1)

        # warm the ACT table set (exp_and_others holds tanh+exp)
        warm = consts.tile([1, 1], f32)
        nc.scalar.activation(warm[:], eye_sb[0:1, 0:1], AF.Tanh)

        ones_row = consts.tile([1, 128], f32)
        nc.vector.memset(ones_row[:], 1.0)


        # decT: [8, 256] -> [128, (dh, b)]
        decT_sb = consts.tile([128, 2, _BL], f32)
        for dh in range(2):
            tp = cx_ps.tile([128, _BL], f32, tag="cx", name=f"decT_ps{dh}")
            nc.tensor.transpose(
                tp[:], dec_sb[:, dh * 128 : (dh + 1) * 128], eye_sb[0:_BL, 0:_BL]
            )
            nc.vector.tensor_copy(decT_sb[:, dh, :], tp[:])
        # dec_proj[i, b] = sum_d w2t[d, i] * decT[d, b]
        dp_sb = consts.tile([128, 2, _BL], f32)  # (ip, ih, b)
        for ih in range(2):
            dpp = cx_ps.tile([128, _BL], f32, tag="cx", name=f"dp_ps{ih}")
            for dh in range(2):
                nc.tensor.matmul(
                    dpp[:],
                    w2t_sb[:, dh, ih * 128 : (ih + 1) * 128],
                    decT_sb[:, dh, :],
                    start=(dh == 0),
                    stop=(dh == 1),
                )
            nc.vector.tensor_copy(dp_sb[:, ih, :], dpp[:])

        # tails assembled on partition-0 / p-major layouts (no scatter DMAs)
        probsF = tail_pool.tile([128, _BL, NTB], f32)  # (p, b, j) -> probs
        ctx_sb = tail_pool.tile([1, _BL, _E], f32)

        # ---------- per-batch stages (emitted software-pipelined) ----------
        encbf_tiles = {}
        tanh_tiles = {}

        def stage_hbm_load(b, parts=2):
            encbf = encbf_pool.tile(
                [128, NTB, _E], bf16, tag="encbf", name=f"encbf_{b}"
            )
            encbf_tiles[b] = encbf
            step = NTB // parts
            for h in range(parts):
                tbs = slice(h * step, (h + 1) * step)
                # SWDGE cast-DMA: f32 HBM -> bf16 SBUF in flight (Pool engine
                # generates descriptors; halves the SBUF-fabric write bytes).
                # Partition p holds 16 consecutive t rows (t = p*16 + j): the
                # DRAM run per partition is 16 KB contiguous -> 64 descriptors
                # per half. Every downstream stage uses this t permutation.
                nc.gpsimd.dma_start(
                    encbf[:, tbs, :],
                    enc.ap()[b].rearrange("(p j) e -> p j e", p=128)[:, tbs, :],
                )

        import os
        tmode = os.environ.get("BASS_TMODE", "pe")

        def stage_transpose(b):
            # PE block transposes (bf16 is_transpose -> PSUM bf16) + DVE
            # copyback. Avoids the DMA xbar entirely: Tile serializes every
            # PASSTHROUGH<->TRANSPOSE xbar-mode switch across all DMA queues
            # (HW deadlock workaround), which throttled the HBM loads.
            encbf = encbf_tiles[b]
            if tmode == "xbar" or (tmode == "hybrid" and b >= 4):
                encT = encT_pool.tile(
                    [128, NTB * 2, 128], bf16, tag="encT", name=f"encT_{b}"
                )
                for h in range(2):
                    tbs = slice(h * HALF_TB, (h + 1) * HALF_TB)
                    nc.sync.dma_start_transpose(
                        encT[:, h * NTB : (h + 1) * NTB, :], encbf[:, tbs, :]
                    )
                return encT
            encT = encT_pool.tile(
                [128, NTB * 2, 128], bf16, tag="encT", name=f"encT_{b}"
            )
            for g in range(4):  # groups of 4 j-blocks (8 [128,128] transposes)
                trp = tr_ps.tile(
                    [128, 8, 128], bf16, tag="tr", name=f"tr_{b}_{g}"
                )
                for k in range(8):
                    j = g * 4 + k // 2
                    eh = k % 2
                    nc.tensor.transpose(
                        trp[:, k, :],
                        encbf[:, j, eh * 128 : (eh + 1) * 128],
                        eyebf_sb,
                    )
                nc.vector.tensor_copy(
                    encT[:, g * 8 : (g + 1) * 8, :], trp[:]
                )
            return encT

        def stage_phase1(b, encT):
            encT_r = encT[:].rearrange("p (tb eh) t -> p eh tb t", eh=2)
            tanh_sb = tanh_pool.tile(
                [128, 2, _T], bf16, tag="tanh", name=f"tanh_{b}"
            )  # (ip, ih, t)
            tanh_tiles[b] = tanh_sb
            for th in range(2):  # 1024-wide halves of t
                for ih in range(2):  # both i-halves per t-half: scores chunks
                    p1 = p1_ps.tile(  # for th become ready one ACT op earlier
                        [128, 1024], f32, tag="p1", name=f"p1_{b}_{ih}_{th}"
                    )
                    for tq in range(2):
                        for eh in range(2):
                            nc.tensor.matmul(
                                p1[:, tq * 512 : (tq + 1) * 512],
                                w1t_sb[:, eh, ih * 128 : (ih + 1) * 128],
                                encT_r[
                                    :, eh, (th * 2 + tq) * 4 : (th * 2 + tq + 1) * 4, :
                                ],
                                start=(eh == 0),
                                stop=(eh == 1),
                            )
                    nc.scalar.activation(
                        tanh_sb[:, ih, th * 1024 : (th + 1) * 1024],
                        p1[:],
                        AF.Tanh,
                        bias=dp_sb[:, ih, b : b + 1],
                    )

        def stage_scores(b):
            tanh_sb = tanh_tiles.pop(b)
            # scores + exp per 512-chunk; u row (bf16, t-ordered) + chunk sums
            urow = small_pool.tile([1, _T], bf16, tag="urow", name=f"urow_{b}")
            urow_tiles[b] = urow
            urow_r = urow[0:1, :].rearrange("o (p j) -> o p j", j=NTB)
            uT = small_pool.tile([128, NTB], bf16, tag="uT", name=f"uT_{b}")
            uT_tiles[b] = uT
            zc = small_pool.tile([1, 4], f32, tag="zc", name=f"zc_{b}")
            zc_tiles[b] = zc
            for tq in range(4):
                sc = cx_ps.tile([1, 512], f32, tag="cx", name=f"sc_{b}_{tq}")
                for ih in range(2):
                    nc.tensor.matmul(
                        sc[:],
                        vt_sb[:, ih, :],
                        tanh_sb[:, ih, tq * 512 : (tq + 1) * 512],
                        start=(ih == 0),
                        stop=(ih == 1),
                    )
                # write u in t-order: chunk element (jl, p) -> t = p*16+4tq+jl
                nc.scalar.activation(
                    urow[0:1, :].rearrange("o (p j) -> o j p", j=NTB)[
                        :, tq * 4 : (tq + 1) * 4, :
                    ],
                    sc[:],
                    AF.Exp,
                    accum_out=zc[:, tq : tq + 1],
                )
                # scatter this chunk to columns right away:
                # uT[p, 4tq+jl] = urow[p*16 + 4tq + jl]. Issued on sync HWDGE
                # (queue is otherwise idle; ~1us lower latency than SWDGE and
                # cross-engine from the exp that writes urow, so race-safe).
                nc.sync.dma_start(
                    uT[:, tq * 4 : (tq + 1) * 4],
                    urow_r[:, :, tq * 4 : (tq + 1) * 4],
                )

        def stage_ctx(b):
            encbf = encbf_tiles.pop(b)
            uT = uT_tiles.pop(b)
            zc = zc_tiles.pop(b)
            urow_tiles.pop(b)
            # 1/Z for this batch
            zb = small_pool.tile([1, 1], f32, tag="zb", name=f"zb_{b}")
            nc.vector.reduce_sum(zb[:], zc[:], axis=mybir.AxisListType.X)
            rz = small_pool.tile([1, 1], f32, tag="rz", name=f"rz_{b}")
            nc.vector.reciprocal(rz[:], zb[:])

            # phase 3: ctx_b = (sum_t u_t * enc_t) * rz
            cps = cx_ps.tile([1, _E], f32, tag="cx", name=f"ctx_ps{b}")
            for tb in range(NTB):
                nc.tensor.matmul(
                    cps[:],
                    uT[:, tb : tb + 1],
                    encbf[:, tb, :],
                    start=(tb == 0),
                    stop=(tb == NTB - 1),
                )
            nc.vector.tensor_scalar_mul(ctx_sb[:, b, :], cps[:], rz[:])

            # probs row for this batch: uT scaled by 1/Z -> (p, j).
            # rz must be per-partition: broadcast via rank-1 PE outer product.
            rzp = cx_ps.tile([128, 1], f32, tag="cx", name=f"rzp_{b}")
            nc.tensor.matmul(rzp[:], ones_row[:], rz[:], start=True, stop=True)
            rz128 = small_pool.tile([128, 1], f32, tag="rz128", name=f"rz128_{b}")
            nc.vector.tensor_copy(rz128[:], rzp[:])
            nc.vector.tensor_scalar_mul(probsF[:, b, :], uT[:], rz128[:])

        # all HBM loads issued upfront (no deps; they prefetch freely),
        # transposes 2 ahead, scores 1 behind, phase3/normalize 2 behind —
        # deep skew so PE's in-order stream never waits on exp/scatter
        urow_tiles = {}
        zc_tiles = {}
        uT_tiles = {}
        for b in range(_BL):
            stage_hbm_load(b, parts=(4 if b == 0 else 2))
        encTs = {}
        for b in range(2):
            encTs[b] = stage_transpose(b)
        for b in range(_BL):
            stage_phase1(b, encTs.pop(b))
            if b >= 1:
                stage_scores(b - 1)
            if b + 2 < _BL:
                encTs[b + 2] = stage_transpose(b + 2)
            if b >= 2:
                stage_ctx(b - 2)
        stage_scores(_BL - 1)
        stage_ctx(_BL - 2)
        stage_ctx(_BL - 1)

        # ---------- output DMAs (probs laid out (p, b, j); t = p*16 + j) ----
        # rows 0..6 go as soon as batch 6's probs are scaled; only the tiny
        # last-row DMA waits on batch 7
        pr = probs_out.ap().rearrange("b (p j) -> p b j", p=128)
        nc.scalar.dma_start(pr[:, 0 : _BL - 1, :], probsF[:, 0 : _BL - 1, :])
        nc.scalar.dma_start(
            pr[:, _BL - 1 : _BL, :], probsF[:, _BL - 1 : _BL, :]
        )
        nc.scalar.dma_start(
            ctx_out.ap()[0 : _BL - 1, :], ctx_sb[:, 0 : _BL - 1, :]
        )
        nc.scalar.dma_start(
            ctx_out.ap()[_BL - 1 : _BL, :], ctx_sb[:, _BL - 1 : _BL, :]
        )



    nc.compile()
    return nc


def _get_nc():
    global _cached
    if _cached is None:
        _cached = _build()
    return _cached


def _host_inputs(w1, w2, v, dec):
    bf = ml_dtypes.bfloat16
    w1t = np.ascontiguousarray(w1.T).astype(bf)  # (e, i)
    w2t = np.ascontiguousarray(w2.T).astype(np.float32)  # (d, i)
    blob = np.zeros((128, _CBLOB_W), dtype=np.uint8)

    def put(col, arr):
        b = np.ascontiguousarray(arr).view(np.uint8).reshape(arr.shape[0], -1)
        blob[: arr.shape[0], col : col + b.shape[1]] = b
        return col + b.shape[1]

    put(0, np.eye(128, dtype=np.float32))
    # dec shard at 512 is per-core; filled below
    put(1536, w2t.reshape(2, 128, _I).transpose(1, 0, 2).copy())  # (dp, dh, i)
    put(3584, w1t.reshape(2, 128, _I).transpose(1, 0, 2).copy())
    put(4608, np.eye(128, dtype=np.float32).astype(bf))
    put(4864, v.reshape(2, 128, 1).astype(bf).transpose(1, 0, 2).copy())
    blobs = []
    for c in range(_NC):
        bc = blob.copy()
        dslice = np.ascontiguousarray(
            dec[c * _BL : (c + 1) * _BL].astype(np.float32)
        )
        bc[:_BL, 512 : 512 + _D * 4] = dslice.view(np.uint8).reshape(_BL, -1)
        blobs.append(bc)
    return blobs


def kernel(encoder_states, decoder_state, w1, w2, v, _want_trace=False):
    from concourse.bass_utils import run_bass_kernel_spmd

    enc = np.ascontiguousarray(encoder_states, dtype=np.float32)
    dec = np.ascontiguousarray(decoder_state, dtype=np.float32)
    blobs = _host_inputs(
        np.asarray(w1, dtype=np.float32),
        np.asarray(w2, dtype=np.float32),
        np.asarray(v, dtype=np.float32),
        dec,
    )

    nc = _get_nc()
    in_maps = []
    for c in range(_NC):
        sl = slice(c * _BL, (c + 1) * _BL)
        in_maps.append({"enc": enc[sl], "cblob": blobs[c]})

    res = run_bass_kernel_spmd(
        nc, in_maps, core_ids=list(range(_NC)), trace=_want_trace
    )
    ctx = np.concatenate([res.results[c]["ctx"] for c in range(_NC)], axis=0)
    probs = np.concatenate([res.results[c]["probs"] for c in range(_NC)], axis=0)
    out = (ctx.astype(np.float32), probs.astype(np.float32)[:, :, None])
    if _want_trace:
        return out, res
    return out


# revision 82
# speedup vs baseline: 1.0044x; 1.0044x over previous
"""Additive attention (Bahdanau) Trainium2 kernel, data-parallel over 8 NeuronCores.

ref:
    enc_proj = einsum('bte,ie->bti', encoder_states, w1)
    dec_proj = einsum('bd,id->bi', decoder_state, w2)
    scores   = einsum('bti,oi->bt', tanh(enc_proj + dec_proj[:,None,:]), v)
    probs    = softmax(scores, axis=1)[:, :, None]
    ctx      = sum(probs * encoder_states, axis=1)
    returns (ctx, probs)

Sharding: batch (64) split 8-way across cores; the small weights are
replicated, host-packed into one constants blob in the transposed layouts the
TensorEngine wants (w1T, w2T, vT, identities).

Per-core dataflow (software-pipelined over the 8 local batches; all stages
emitted with explicit skews so each engine's in-order stream never stalls):
  - all HBM loads upfront: gpsimd SWDGE cast-DMA f32->bf16 in flight, with
    partition p holding 16 consecutive t rows (t = p*16 + j) so each
    partition's DRAM run is 16 KB contiguous (64 descriptors per half-batch)
  - transposes (2 batches ahead): PE is_transpose matmuls (bf16 -> PSUM-bf16
    blocks) + DVE copyback -> encT [e, t]. The DMA xbar is deliberately NOT
    used: Tile serializes every PASSTHROUGH<->TRANSPOSE xbar-mode switch
    across all queues (HW deadlock workaround) which throttles the loads.
  - phase1: PE bf16 matmuls (w1T stationary, encT moving) -> PSUM [i, t]
  - ScalarE tanh(psum + dec_proj[:, b] bias) -> SBUF bf16 [i, t]
  - scores (1 batch behind): PE matmul vT x tanh -> PSUM [1, 512] chunks;
    ScalarE exp writes the u row bf16 in t-order via a strided AP
    (+ accum_out chunk sums). |scores| < 2 so no max subtraction needed.
    Each chunk is immediately scattered to uT columns [t128, j] by a plain
    reshape DMA on the otherwise-idle sync HWDGE queue (contiguous runs).
  - phase3 (2 batches behind): PE matmul uT-column x enc-bf16 block, PSUM
    accumulated over the 16 t-blocks; DVE scales by 1/Z (normalize late)
    into ctx and probs staging; two output DMAs at the end.

Queue discipline (hard-won): gpsimd SWDGE = enc loads only (its FIFO must
never contain compute-gated DMAs); sync HWDGE = everything latency-critical
(consts, uT scatters, last output rows — lowest fixed latency, idle queue);
scalar = bulk output rows. A DMA must never be issued from the engine that
computes its source (same-engine issue emits no semaphore and races the ACT
pipeline on silicon).
"""

import numpy as np
import ml_dtypes

_B, _T, _E, _I, _D = 64, 2048, 256, 256, 256
_CBLOB_W = 4868
_NC = 8
_BL = _B // _NC  # batches per core

_cached = None


def _build():
    from contextlib import ExitStack

    import concourse.bass as bass
    import concourse.tile as tile
    from concourse import bacc, mybir

    f32 = mybir.dt.float32
    bf16 = mybir.dt.bfloat16
    AF = mybir.ActivationFunctionType

    nc = bacc.Bacc(
        "TRN2",
        target_bir_lowering=False,
        debug=False,
        num_devices=_NC,
        dynamic_dma_scratch_size=65536,
        num_swdge_queues=4,
    )

    u8 = mybir.dt.uint8
    enc = nc.declare_dram_parameter("enc", [_BL, _T, _E], f32, isOutput=False)
    cblob = nc.declare_dram_parameter("cblob", [128, _CBLOB_W], u8, isOutput=False)
    ctx_out = nc.declare_dram_parameter("ctx", [_BL, _E], f32, isOutput=True)
    probs_out = nc.declare_dram_parameter("probs", [_BL, _T], f32, isOutput=True)

    NTB = _T // 128  # 16 t-blocks per batch
    HALF_TB = NTB // 2  # load/cast/transpose granularity

    with tile.TileContext(nc) as tc, ExitStack() as ex:
        pool = lambda name, bufs, **kw: ex.enter_context(
            tc.tile_pool(name=name, bufs=bufs, **kw)
        )
        consts = pool("consts", 1)
        encbf_pool = pool("encbf", _BL)  # batch b's tile lives until phase3(b)
        encT_pool = pool("encT", 3)
        tanh_pool = pool("tanh", 2)
        small_pool = pool("small", 4)
        tail_pool = pool("tail", 1)
        p1_ps = pool("p1ps", 2, space="PSUM")  # [128, 1024] f32 -> 2 banks each
        tr_ps = pool("trps", 1, space="PSUM")  # [128, 8, 128] bf16 -> 1 bank each
        cx_ps = pool("cxps", 3, space="PSUM")  # scores/ctx/misc -> 1 bank each

        # ---------- constants: one host-packed blob, two DMAs (the first
        # carries just eye+dec so the dec-transpose preamble starts early) ----
        blob = consts.tile([128, _CBLOB_W], u8)
        nc.sync.dma_start(blob[:, 0:1536], cblob.ap()[:, 0:1536])
        nc.sync.dma_start(blob[:, 1536:], cblob.ap()[:, 1536:])
        eye_sb = blob[:, 0:512].bitcast(f32)  # [128, 128]
        dec_sb = blob[0:_BL, 512:1536].bitcast(f32)  # [8, 256]
        w2t_sb = blob[:, 1536:3584].bitcast(f32).rearrange(
            "p (dh i) -> p dh i", dh=2
        )  # (dp, dh, i)
        w1t_sb = blob[:, 3584:4608].bitcast(bf16).rearrange(
            "p (eh i) -> p eh i", eh=2
        )  # (ep, eh, i)
        eyebf_sb = blob[:, 4608:4864].bitcast(bf16)  # [128, 128]
        vt_sb = blob[:, 4864:4868].bitcast(bf16).rearrange(
            "p (ih o) -> p ih o", ih=2
        )  # (ip, ih, 1)

        # warm the ACT table set (exp_and_others holds tanh+exp)
        warm = consts.tile([1, 1], f32)
        nc.scalar.activation(warm[:], eye_sb[0:1, 0:1], AF.Tanh)

        ones_row = consts.tile([1, 128], f32)
        nc.vector.memset(ones_row[:], 1.0)


        # decT: [8, 256] -> [128, (dh, b)]
        decT_sb = consts.tile([128, 2, _BL], f32)
        for dh in range(2):
            tp = cx_ps.tile([128, _BL], f32, tag="cx", name=f"decT_ps{dh}")
            nc.tensor.transpose(
                tp[:], dec_sb[:, dh * 128 : (dh + 1) * 128], eye_sb[0:_BL, 0:_BL]
            )
            nc.vector.tensor_copy(decT_sb[:, dh, :], tp[:])
        # dec_proj[i, b] = sum_d w2t[d, i] * decT[d, b]
        dp_sb = consts.tile([128, 2, _BL], f32)  # (ip, ih, b)
        for ih in range(2):
            dpp = cx_ps.tile([128, _BL], f32, tag="cx", name=f"dp_ps{ih}")
            for dh in range(2):
                nc.tensor.matmul(
                    dpp[:],
                    w2t_sb[:, dh, ih * 128 : (ih + 1) * 128],
                    decT_sb[:, dh, :],
                    start=(dh == 0),
                    stop=(dh == 1),
                )
            nc.vector.tensor_copy(dp_sb[:, ih, :], dpp[:])

        # tails assembled on partition-0 / p-major layouts (no scatter DMAs)
        probsF = tail_pool.tile([128, _BL, NTB], f32)  # (p, b, j) -> probs
        ctx_sb = tail_pool.tile([1, _BL, _E], f32)

        # ---------- per-batch stages (emitted software-pipelined) ----------
        encbf_tiles = {}
        tanh_tiles = {}

        def stage_hbm_load(b, parts=2):
            encbf = encbf_pool.tile(
                [128, NTB, _E], bf16, tag="encbf", name=f"encbf_{b}"
            )
            encbf_tiles[b] = encbf
            step = NTB // parts
            for h in range(parts):
                tbs = slice(h * step, (h + 1) * step)
                # SWDGE cast-DMA: f32 HBM -> bf16 SBUF in flight (Pool engine
                # generates descriptors; halves the SBUF-fabric write bytes).
                # Partition p holds 16 consecutive t rows (t = p*16 + j): the
                # DRAM run per partition is 16 KB contiguous -> 64 descriptors
                # per half. Every downstream stage uses this t permutation.
                nc.gpsimd.dma_start(
                    encbf[:, tbs, :],
                    enc.ap()[b].rearrange("(p j) e -> p j e", p=128)[:, tbs, :],
                )

        import os
        tmode = os.environ.get("BASS_TMODE", "pe")

        def stage_transpose(b):
            # PE block transposes (bf16 is_transpose -> PSUM bf16) + DVE
            # copyback. Avoids the DMA xbar entirely: Tile serializes every
            # PASSTHROUGH<->TRANSPOSE xbar-mode switch across all DMA queues
            # (HW deadlock workaround), which throttled the HBM loads.
            encbf = encbf_tiles[b]
            if tmode == "xbar" or (tmode == "hybrid" and b >= 4):
                encT = encT_pool.tile(
                    [128, NTB * 2, 128], bf16, tag="encT", name=f"encT_{b}"
                )
                for h in range(2):
                    tbs = slice(h * HALF_TB, (h + 1) * HALF_TB)
                    nc.sync.dma_start_transpose(
                        encT[:, h * NTB : (h + 1) * NTB, :], encbf[:, tbs, :]
                    )
                return encT
            encT = encT_pool.tile(
                [128, NTB * 2, 128], bf16, tag="encT", name=f"encT_{b}"
            )
            for g in range(4):  # groups of 4 j-blocks (8 [128,128] transposes)
                trp = tr_ps.tile(
                    [128, 8, 128], bf16, tag="tr", name=f"tr_{b}_{g}"
                )
                for k in range(8):
                    j = g * 4 + k // 2
                    eh = k % 2
                    nc.tensor.transpose(
                        trp[:, k, :],
                        encbf[:, j, eh * 128 : (eh + 1) * 128],
                        eyebf_sb,
                    )
                nc.vector.tensor_copy(
                    encT[:, g * 8 : (g + 1) * 8, :], trp[:]
                )
            return encT

        def stage_phase1(b, encT):
            encT_r = encT[:].rearrange("p (tb eh) t -> p eh tb t", eh=2)
            tanh_sb = tanh_pool.tile(
                [128, 2, _T], bf16, tag="tanh", name=f"tanh_{b}"
            )  # (ip, ih, t)
            tanh_tiles[b] = tanh_sb
            for th in range(2):  # 1024-wide halves of t
                for ih in range(2):  # both i-halves per t-half: scores chunks
                    p1 = p1_ps.tile(  # for th become ready one ACT op earlier
                        [128, 1024], f32, tag="p1", name=f"p1_{b}_{ih}_{th}"
                    )
                    for tq in range(2):
                        for eh in range(2):
                            nc.tensor.matmul(
                                p1[:, tq * 512 : (tq + 1) * 512],
                                w1t_sb[:, eh, ih * 128 : (ih + 1) * 128],
                                encT_r[
                                    :, eh, (th * 2 + tq) * 4 : (th * 2 + tq + 1) * 4, :
                                ],
                                start=(eh == 0),
                                stop=(eh == 1),
                            )
                    nc.scalar.activation(
                        tanh_sb[:, ih, th * 1024 : (th + 1) * 1024],
                        p1[:],
                        AF.Tanh,
                        bias=dp_sb[:, ih, b : b + 1],
                    )

        def stage_scores(b):
            tanh_sb = tanh_tiles.pop(b)
            # scores + exp per 512-chunk; u row (bf16, t-ordered) + chunk sums
            urow = small_pool.tile([1, _T], bf16, tag="urow", name=f"urow_{b}")
            urow_tiles[b] = urow
            urow_r = urow[0:1, :].rearrange("o (p j) -> o p j", j=NTB)
            uT = small_pool.tile([128, NTB], bf16, tag="uT", name=f"uT_{b}")
            uT_tiles[b] = uT
            zc = small_pool.tile([1, 4], f32, tag="zc", name=f"zc_{b}")
            zc_tiles[b] = zc
            for tq in range(4):
                sc = cx_ps.tile([1, 512], f32, tag="cx", name=f"sc_{b}_{tq}")
                for ih in range(2):
                    nc.tensor.matmul(
                        sc[:],
                        vt_sb[:, ih, :],
                        tanh_sb[:, ih, tq * 512 : (tq + 1) * 512],
                        start=(ih == 0),
                        stop=(ih == 1),
                    )
                # write u in t-order: chunk element (jl, p) -> t = p*16+4tq+jl
                nc.scalar.activation(
                    urow[0:1, :].rearrange("o (p j) -> o j p", j=NTB)[
                        :, tq * 4 : (tq + 1) * 4, :
                    ],
                    sc[:],
                    AF.Exp,
                    accum_out=zc[:, tq : tq + 1],
                )
                # scatter this chunk to columns right away:
                # uT[p, 4tq+jl] = urow[p*16 + 4tq + jl]. Issued on sync HWDGE
                # (queue is otherwise idle; ~1us lower latency than SWDGE and
                # cross-engine from the exp that writes urow, so race-safe).
                nc.sync.dma_start(
                    uT[:, tq * 4 : (tq + 1) * 4],
                    urow_r[:, :, tq * 4 : (tq + 1) * 4],
                )

        def stage_ctx(b):
            encbf = encbf_tiles.pop(b)
            uT = uT_tiles.pop(b)
            zc = zc_tiles.pop(b)
            urow_tiles.pop(b)
            # 1/Z for this batch
            zb = small_pool.tile([1, 1], f32, tag="zb", name=f"zb_{b}")
            nc.vector.reduce_sum(zb[:], zc[:], axis=mybir.AxisListType.X)
            rz = small_pool.tile([1, 1], f32, tag="rz", name=f"rz_{b}")
            nc.vector.reciprocal(rz[:], zb[:])

            # phase 3: ctx_b = (sum_t u_t * enc_t) * rz
            cps = cx_ps.tile([1, _E], f32, tag="cx", name=f"ctx_ps{b}")
            for tb in range(NTB):
                nc.tensor.matmul(
                    cps[:],
                    uT[:, tb : tb + 1],
                    encbf[:, tb, :],
                    start=(tb == 0),
                    stop=(tb == NTB - 1),
                )
            nc.vector.tensor_scalar_mul(ctx_sb[:, b, :], cps[:], rz[:])

            # probs row for this batch: uT scaled by 1/Z -> (p, j).
            # rz must be per-partition: broadcast via rank-1 PE outer product.
            rzp = cx_ps.tile([128, 1], f32, tag="cx", name=f"rzp_{b}")
            nc.tensor.matmul(rzp[:], ones_row[:], rz[:], start=True, stop=True)
            rz128 = small_pool.tile([128, 1], f32, tag="rz128", name=f"rz128_{b}")
            nc.vector.tensor_copy(rz128[:], rzp[:])
            nc.vector.tensor_scalar_mul(probsF[:, b, :], uT[:], rz128[:])

        # all HBM loads issued upfront (no deps; they prefetch freely),
        # transposes 2 ahead, scores 1 behind, phase3/normalize 2 behind —
        # deep skew so PE's in-order stream never waits on exp/scatter
        urow_tiles = {}
        zc_tiles = {}
        uT_tiles = {}
        for b in range(_BL):
            stage_hbm_load(b, parts=(4 if b == 0 else 2))
        encTs = {}
        for b in range(2):
            encTs[b] = stage_transpose(b)
        for b in range(_BL):
            stage_phase1(b, encTs.pop(b))
            if b >= 1:
                stage_scores(b - 1)
            if b + 2 < _BL:
                encTs[b + 2] = stage_transpose(b + 2)
            if b >= 2:
                stage_ctx(b - 2)
        stage_scores(_BL - 1)
        stage_ctx(_BL - 2)
        stage_ctx(_BL - 1)

        # ---------- output DMAs (probs laid out (p, b, j); t = p*16 + j) ----
        # rows 0..6 go as soon as batch 6's probs are scaled; only the tiny
        # last-row DMA waits on batch 7
        pr = probs_out.ap().rearrange("b (p j) -> p b j", p=128)
        nc.scalar.dma_start(pr[:, 0 : _BL - 1, :], probsF[:, 0 : _BL - 1, :])
        nc.sync.dma_start(
            pr[:, _BL - 1 : _BL, :], probsF[:, _BL - 1 : _BL, :]
        )
        nc.scalar.dma_start(
            ctx_out.ap()[0 : _BL - 1, :], ctx_sb[:, 0 : _BL - 1, :]
        )
        nc.sync.dma_start(
            ctx_out.ap()[_BL - 1 : _BL, :], ctx_sb[:, _BL - 1 : _BL, :]
        )



    nc.compile()
    return nc


def _get_nc():
    global _cached
    if _cached is None:
        _cached = _build()
    return _cached


def _host_inputs(w1, w2, v, dec):
    bf = ml_dtypes.bfloat16
    w1t = np.ascontiguousarray(w1.T).astype(bf)  # (e, i)
    w2t = np.ascontiguousarray(w2.T).astype(np.float32)  # (d, i)
    blob = np.zeros((128, _CBLOB_W), dtype=np.uint8)

    def put(col, arr):
        b = np.ascontiguousarray(arr).view(np.uint8).reshape(arr.shape[0], -1)
        blob[: arr.shape[0], col : col + b.shape[1]] = b
        return col + b.shape[1]

    put(0, np.eye(128, dtype=np.float32))
    # dec shard at 512 is per-core; filled below
    put(1536, w2t.reshape(2, 128, _I).transpose(1, 0, 2).copy())  # (dp, dh, i)
    put(3584, w1t.reshape(2, 128, _I).transpose(1, 0, 2).copy())
    put(4608, np.eye(128, dtype=np.float32).astype(bf))
    put(4864, v.reshape(2, 128, 1).astype(bf).transpose(1, 0, 2).copy())
    blobs = []
    for c in range(_NC):
        bc = blob.copy()
        dslice = np.ascontiguousarray(
            dec[c * _BL : (c + 1) * _BL].astype(np.float32)
        )
        bc[:_BL, 512 : 512 + _D * 4] = dslice.view(np.uint8).reshape(_BL, -1)
        blobs.append(bc)
    return blobs


def kernel(encoder_states, decoder_state, w1, w2, v, _want_trace=False):
    from concourse.bass_utils import run_bass_kernel_spmd

    enc = np.ascontiguousarray(encoder_states, dtype=np.float32)
    dec = np.ascontiguousarray(decoder_state, dtype=np.float32)
    blobs = _host_inputs(
        np.asarray(w1, dtype=np.float32),
        np.asarray(w2, dtype=np.float32),
        np.asarray(v, dtype=np.float32),
        dec,
    )

    nc = _get_nc()
    in_maps = []
    for c in range(_NC):
        sl = slice(c * _BL, (c + 1) * _BL)
        in_maps.append({"enc": enc[sl], "cblob": blobs[c]})

    res = run_bass_kernel_spmd(
        nc, in_maps, core_ids=list(range(_NC)), trace=_want_trace
    )
    ctx = np.concatenate([res.results[c]["ctx"] for c in range(_NC)], axis=0)
    probs = np.concatenate([res.results[c]["probs"] for c in range(_NC)], axis=0)
    out = (ctx.astype(np.float32), probs.astype(np.float32)[:, :, None])
    if _want_trace:
        return out, res
    return out
